# revision 2
# baseline (speedup 1.0000x reference)
"""Trainium2 Bass kernel for the Lorenz-63 square-root EKF benchmark.

Problem: N=1024 independent trajectories, each filtered over Ty=1024 steps
with tiny 3x3 state-space matrices.  Sharding: data-parallel over the N axis
— 8 NeuronCores x 128 trajectories, one trajectory per SBUF partition; the
sequential time recursion runs on-chip with ~27 small instructions per step
spread across the Vector/Scalar/GPSIMD engines.

The reference's QR-based square-root update is replaced by the algebraically
equivalent (and sign-invariant) covariance recursion, which simplifies
drastically because H = I, R = r*I, Q = q2*I:

    x_neg = f(x)                      # Euler-discretized Lorenz-63
    Re    = F P F^T + (q2 + r) I      # innovation covariance
    K     = I - r * inv(Re)           # Kalman gain
    P_new = r * K                     # posterior covariance (exactly symmetric)
    x_new = x_neg + K (y - x_neg)

inv(Re) is computed with the closed-form 3x3 adjugate (cyclic cofactors) and
a single reciprocal.  The filter recursion is contractive, so fp32 rounding
differences vs. the reference stay at the ~1e-6 level over all 1024 steps.
"""

import numpy as np

import concourse.bass as bass
import concourse.tile as tile
from concourse import bacc, mybir

F32 = mybir.dt.float32
ALU = mybir.AluOpType

N_CORES = 8
N_TRAJ = 1024
TY = 1024
P = N_TRAJ // N_CORES  # 128 trajectories per core == SBUF partitions

DELTA = np.float32(0.02)
SIGMA = np.float32(10.0)
RHO = np.float32(28.0)
BETA = np.float32(8.0 / 3.0)


def _make_consts() -> np.ndarray:
    """[0:15] A (3x5): x_neg_i = sum_k A[i,k]*w_k, w=[x0,x1,x2,x0x1,x0x2];
    [15:17] coeffs for F12,F20; [17:26] I3 row-major."""
    A = np.zeros((3, 5), np.float32)
    A[0, 0] = 1 - DELTA * SIGMA
    A[0, 1] = DELTA * SIGMA
    A[1, 0] = DELTA * RHO
    A[1, 1] = 1 - DELTA
    A[1, 4] = -DELTA
    A[2, 2] = 1 - DELTA * BETA
    A[2, 3] = DELTA
    cf2 = np.array([-DELTA, DELTA], np.float32)
    i9 = np.eye(3, dtype=np.float32).reshape(-1)
    return np.concatenate([A.reshape(-1), cf2, i9]).astype(np.float32)


def _ap(base: bass.AP, off: int, dims) -> bass.AP:
    """Custom free-axis access pattern on a tile; keeps the partition dim."""
    return bass.AP(
        tensor=base.tensor,
        offset=base.offset + off,
        ap=[list(base.ap[0])] + [list(d) for d in dims],
    )


def _build_nc(Ty: int, r2: float, q2: float):
    cdiag = float(np.float32(q2) + np.float32(r2))
    nc = bacc.Bacc(
        "TRN2", target_bir_lowering=False, debug=False, enable_asserts=False
    )
    y_dram = nc.dram_tensor("y_in", [P, Ty, 3], F32, kind="ExternalInput")
    cst_dram = nc.dram_tensor("cst_in", [26], F32, kind="ExternalInput")
    xs_dram = nc.dram_tensor("xs_out", [P, Ty + 1, 3], F32, kind="ExternalOutput")
    ps_dram = nc.dram_tensor("ps_out", [P, Ty + 1, 3, 3], F32, kind="ExternalOutput")

    with tile.TileContext(nc) as tc:
        with (
            tc.tile_pool(name="big", bufs=1) as big,
            tc.tile_pool(name="tmp", bufs=3) as tmp,
        ):
            y_buf = big.tile([P, Ty * 3], F32)
            xs_buf = big.tile([P, (Ty + 1) * 3], F32)
            ps_buf = big.tile([P, (Ty + 1) * 9], F32)
            cst = big.tile([P, 26], F32)
            f9 = big.tile([P, 9], F32)
            w = big.tile([P, 5], F32)

            nc.sync.dma_start(
                out=y_buf[:, :], in_=y_dram.ap().rearrange("p t d -> p (t d)")
            )
            cst_b = bass.AP(tensor=cst_dram, offset=0, ap=[[0, P], [1, 26]])
            nc.sync.dma_start(out=cst[:, :], in_=cst_b)

            nc.vector.memset(w[:, 0:3], 1.0)               # x0 = ones
            nc.vector.memset(ps_buf[:, 0:9], 0.0)          # P0 = I in slot 0
            nc.vector.memset(_ap(ps_buf[:], 0, [[4, 3]]), 1.0)
            nc.vector.memset(xs_buf[:, 0:3], 0.0)
            nc.gpsimd.memset(f9[:, 0:1], float(1 - DELTA * SIGMA))
            nc.gpsimd.memset(f9[:, 1:2], float(DELTA * SIGMA))
            nc.gpsimd.memset(f9[:, 2:3], 0.0)
            nc.gpsimd.memset(f9[:, 4:5], float(1 - DELTA))
            nc.gpsimd.memset(f9[:, 8:9], float(1 - DELTA * BETA))

            cA15 = _ap(cst[:], 0, [[5, 3], [1, 5]])
            cF2 = cst[:, 15:17]
            cI9 = cst[:, 17:26]

            for t in range(Ty):
                p_next = _ap(ps_buf[:], 9 * (t + 1), [[1, 9]])
                y_t = y_buf[:, 3 * t : 3 * t + 3]
                xs_next = xs_buf[:, 3 * (t + 1) : 3 * (t + 1) + 3]

                t15 = tmp.tile([P, 3, 5], F32, tag="t15")
                t27a = tmp.tile([P, 3, 3, 3], F32, tag="t27a")
                t27b = tmp.tile([P, 3, 3, 3], F32, tag="t27b")
                fp9 = tmp.tile([P, 9], F32, tag="fp9")
                re9 = tmp.tile([P, 9], F32, tag="re9")
                rq = tmp.tile([P, 30], F32, tag="rq")
                adj9 = tmp.tile([P, 9], F32, tag="adj9")
                t9m = tmp.tile([P, 3, 3], F32, tag="t9m")
                x_neg = tmp.tile([P, 3], F32, tag="x_neg")
                innov = tmp.tile([P, 3], F32, tag="innov")
                dx3 = tmp.tile([P, 3], F32, tag="dx3")
                k9 = tmp.tile([P, 9], F32, tag="k9")
                dets = tmp.tile([P, 1], F32, tag="dets")
                s1 = tmp.tile([P, 1], F32, tag="s1")
                waste3 = tmp.tile([P, 3], F32, tag="waste3")

                # x_neg = f(x_prev), with w = [x0, x1, x2, x0x1, x0x2]
                nc.vector.tensor_mul(w[:, 3:5], _ap(w[:], 0, [[0, 2]]), w[:, 1:3])
                nc.vector.tensor_mul(t15[:], _ap(w[:], 0, [[0, 3], [1, 5]]), cA15)
                nc.vector.reduce_sum(x_neg[:, :], t15[:], axis=mybir.AxisListType.X)

                # F variable entries: F10, F12, F20, F21
                nc.gpsimd.tensor_scalar(
                    f9[:, 3:4], w[:, 2:3], -float(DELTA), float(DELTA * RHO),
                    op0=ALU.mult, op1=ALU.add,
                )
                nc.gpsimd.tensor_mul(f9[:, 5:7], w[:, 0:2], cF2)
                nc.gpsimd.tensor_scalar(
                    f9[:, 7:8], w[:, 0:1], float(DELTA), None, op0=ALU.mult
                )

                # Re = F P F^T + (q2+r) I via two (i,j,k) product+reduce pairs
                f_ik = _ap(f9[:], 0, [[3, 3], [0, 3], [1, 3]])
                p_kj = _ap(ps_buf[:], 9 * t, [[0, 3], [1, 3], [3, 3]])
                nc.vector.tensor_mul(t27a[:], f_ik, p_kj)
                nc.vector.reduce_sum(
                    _ap(fp9[:], 0, [[3, 3], [1, 3]]), t27a[:],
                    axis=mybir.AxisListType.X,
                )
                fp_ik = _ap(fp9[:], 0, [[3, 3], [0, 3], [1, 3]])
                f_jk = _ap(f9[:], 0, [[0, 3], [3, 3], [1, 3]])
                nc.vector.tensor_mul(t27b[:], fp_ik, f_jk)
                nc.vector.reduce_sum(
                    _ap(re9[:], 0, [[3, 3], [1, 3]]), t27b[:],
                    axis=mybir.AxisListType.X,
                )
                nc.gpsimd.tensor_scalar(
                    _ap(re9[:], 0, [[4, 3]]), _ap(re9[:], 0, [[4, 3]]),
                    cdiag, None, op0=ALU.add,
                )

                # Rquad: 5 column-blocks of Re, each column doubled
                re_cp = _ap(re9[:], 0, [[1, 3], [3, 3]])
                re_cp2 = _ap(re9[:], 0, [[1, 2], [3, 3]])
                nc.scalar.copy(_ap(rq[:], 0, [[6, 3], [1, 3]]), re_cp)
                nc.scalar.copy(_ap(rq[:], 3, [[6, 3], [1, 3]]), re_cp)
                nc.scalar.copy(_ap(rq[:], 18, [[6, 2], [1, 3]]), re_cp2)
                nc.scalar.copy(_ap(rq[:], 21, [[6, 2], [1, 3]]), re_cp2)

                # adjugate via cyclic cofactors
                adj_out = _ap(adj9[:], 0, [[3, 3], [1, 3]])
                nc.vector.tensor_mul(
                    adj_out,
                    _ap(rq[:], 7, [[1, 3], [6, 3]]),
                    _ap(rq[:], 14, [[1, 3], [6, 3]]),
                )
                nc.vector.tensor_mul(
                    _ap(t9m[:], 0, [[3, 3], [1, 3]]),
                    _ap(rq[:], 13, [[1, 3], [6, 3]]),
                    _ap(rq[:], 8, [[1, 3], [6, 3]]),
                )
                nc.vector.tensor_sub(
                    adj9[:, :], adj9[:, :], t9m[:].rearrange("p a b -> p (a b)")
                )

                # s = -r/det  (det via row-0 cofactor expansion)
                nc.vector.scalar_tensor_tensor(
                    waste3[:, :], re9[:, 0:3], -1.0 / float(r2), adj9[:, 0:3],
                    op0=ALU.mult, op1=ALU.mult,
                )
                nc.vector.reduce_sum(
                    dets[:, :], waste3[:, :], axis=mybir.AxisListType.X
                )
                nc.vector.reciprocal(s1[:, :], dets[:, :])

                # K = adj*s + I ;  P_new = r*K
                nc.vector.scalar_tensor_tensor(
                    k9[:, :], adj9[:, :], s1[:, :], cI9, op0=ALU.mult, op1=ALU.add
                )
                nc.gpsimd.tensor_scalar(
                    p_next, k9[:, :], float(r2), None, op0=ALU.mult
                )

                # x_new = x_neg + K (y - x_neg)
                nc.gpsimd.tensor_sub(innov[:, :], y_t, x_neg[:, :])
                nc.vector.tensor_mul(
                    t9m[:],
                    _ap(k9[:], 0, [[3, 3], [1, 3]]),
                    _ap(innov[:], 0, [[0, 3], [1, 3]]),
                )
                nc.vector.reduce_sum(dx3[:, :], t9m[:], axis=mybir.AxisListType.X)
                nc.vector.tensor_add(w[:, 0:3], x_neg[:, :], dx3[:, :])
                nc.scalar.copy(xs_next, w[:, 0:3])

            nc.vector.memset(ps_buf[:, 0:9], 0.0)  # output slot 0 = zeros

            nc.sync.dma_start(
                out=xs_dram.ap().rearrange("p t d -> p (t d)"), in_=xs_buf[:, :]
            )
            nc.sync.dma_start(
                out=ps_dram.ap().rearrange("p t a b -> p (t a b)"), in_=ps_buf[:, :]
            )

    nc.compile()
    return nc


def _build_nc_trivial():
    """Same I/O shapes, no compute — times dispatch + transfer overhead."""
    nc = bacc.Bacc(
        "TRN2", target_bir_lowering=False, debug=False, enable_asserts=False
    )
    y_dram = nc.dram_tensor("y_in", [P, TY, 3], F32, kind="ExternalInput")
    cst_dram = nc.dram_tensor("cst_in", [26], F32, kind="ExternalInput")
    xs_dram = nc.dram_tensor("xs_out", [P, TY + 1, 3], F32, kind="ExternalOutput")
    ps_dram = nc.dram_tensor("ps_out", [P, TY + 1, 3, 3], F32, kind="ExternalOutput")
    with tile.TileContext(nc) as tc:
        with tc.tile_pool(name="big", bufs=1) as big:
            xs_buf = big.tile([P, (TY + 1) * 3], F32)
            ps_buf = big.tile([P, (TY + 1) * 9], F32)
            cst = big.tile([P, 26], F32)
            cst_b = bass.AP(tensor=cst_dram, offset=0, ap=[[0, P], [1, 26]])
            nc.sync.dma_start(out=cst[:, :], in_=cst_b)
            nc.vector.memset(xs_buf[:, :], 0.0)
            nc.vector.memset(ps_buf[:, :], 0.0)
            nc.sync.dma_start(
                out=xs_dram.ap().rearrange("p t d -> p (t d)"), in_=xs_buf[:, :]
            )
            nc.sync.dma_start(
                out=ps_dram.ap().rearrange("p t a b -> p (t a b)"), in_=ps_buf[:, :]
            )
    nc.compile()
    return nc


_NC_CACHE: dict = {}


def kernel(X: np.ndarray, Y: np.ndarray, Q: np.ndarray, R: np.ndarray):
    X = np.asarray(X)
    Y = np.asarray(Y)
    r2 = float(np.asarray(R)[0, 0])
    q2 = float(np.asarray(Q)[0, 0])
    N, Ty, _ = Y.shape
    assert N == N_TRAJ and Ty == TY, (N, Ty)

    key = (Ty, r2, q2)
    if key not in _NC_CACHE:
        _NC_CACHE[key] = _build_nc(Ty, r2, q2)
    nc = _NC_CACHE[key]

    from concourse.bass_utils import run_bass_kernel_spmd

    cst = _make_consts()
    in_maps = [
        {"y_in": np.ascontiguousarray(Y[i * P : (i + 1) * P]), "cst_in": cst}
        for i in range(N_CORES)
    ]
    res = run_bass_kernel_spmd(nc, in_maps, core_ids=list(range(N_CORES)))

    traj = np.concatenate(
        [res.results[i]["xs_out"] for i in range(N_CORES)], axis=0
    ).astype(np.float32)
    Pk = np.concatenate(
        [res.results[i]["ps_out"] for i in range(N_CORES)], axis=0
    ).astype(np.float32)

    mse_arr = np.mean(
        (X[:, 1:, :].astype(np.float32) - traj[:, 1:, :]) ** 2,
        axis=(1, 2), dtype=np.float32,
    )
    mse_db = np.float32(
        np.mean(np.float32(10.0) * np.log10(mse_arr), dtype=np.float32)
    )
    return traj, Pk, mse_db
